# revision 19
# baseline (speedup 1.0000x reference)
"""Multi-head attention (B=2, S=2048, H=1024, 16 heads) on 8 NeuronCores.

Tensor-parallel sharding: 2 heads per core.  Each core computes QKV for its
heads, full attention over the sequence for its heads, and a partial output
projection (its 128 rows of w_dense).  The host sums the 8 partial outputs
(the all-reduce) and adds the output-side bias terms.

v5 structure (single fused phase):
  * hs is transposed on the host and shipped as bf16 [HID, SEQ]; QKV
    weights and Q/K tiles in bf16 (Q/K stationaries get fast weight load).
  * K-bias dropped on device (softmax-invariant, exact); V/dense biases
    commute to the host.
  * ctx is normalized (divided by the softmax row sums) BEFORE the output
    projection (rowsums broadcast by a 1-row PE matmul, fast approximate
    reciprocal, in-place multiply), so both heads' dense contributions
    collapse into a single matmul per output tile.
  * The QKV projection is fused into the attention loop: only windows 0-3
    (batch 0) run up front; windows 4-7 are converted to generators and
    their matmul chains / V transposes are injected into the attention
    loop of batch 0's blocks, filling the PE slack under the scalar(EXP)
    pace.  Dense matmuls of block N ride block N+1 as before, and the
    final flush splits evictions between Scalar and Vector and rotates
    dense PSUM through the idle score pool.
  * One PSUM budget for everything (8 banks): score ring 2x[128,1024]
    (4 banks) + PV accumulators 2x[65,512] (2) + a shared background ring
    2x[128,512] (2) used by QKV chains, V transposes (f32r via bitcast),
    dense steps and the rowsum broadcasts.

Layout notes (per core), all PE matmuls in plain 128x128 mode:
  hsw  [128, hid/128, 512]  bf16 window of host-pretransposed hs.
  QTz/KTz [128, h, seq] q/k transposed per head (bf16), zero-padded to a
                        full 128-partition contraction (rows 64-127 = 0).
  Vn  [128, 32, 2, 66]  v natural: partition = seq within 128-chunk,
                        [chunk, head, dim]; col 64 is 1.0 so the P@V
                        matmul also emits the softmax denominators.
  PT  [128, RING, 1024] exp(scores) ring: partition = k within chunk.
  ctxT [128, seq]       context transposed, head 0 rows 0-63 and head 1
                        rows 64-127, so one dense matmul contracts both
                        heads against full-width w_dense slices.
"""

import os
import sys
import types
from collections import deque

# Reset cores at runtime init: a device left in a degraded state by a
# previous run otherwise inflates kernel time by ~20% (observed
# repeatedly during tuning).  Must be set before the runtime loads.
os.environ.setdefault("NEURON_RT_RESET_CORES", "1")

sys.path.insert(0, "/opt/trn_rl_repo")

import numpy as np

try:
    import ml_dtypes

    BF16_NP = ml_dtypes.bfloat16
except ImportError:  # pragma: no cover
    BF16_NP = None


def _install_ntff_shim():
    """The trimmed container image lacks ``antenv.axon_hooks``, which
    ``run_bass_kernel_spmd(trace=True)`` needs to capture NTFF profiles
    under axon.  Recreate it from the boot helper + the injected .so."""
    if "antenv.axon_hooks" in sys.modules:
        return
    try:
        from trn_agent_boot.trn_boot import _ntff_profile_via_ctypes
        so = "/opt/axon/libaxon_pjrt.so"
        if not os.path.exists(so):
            return
        hook = _ntff_profile_via_ctypes(so)
        mod = types.ModuleType("antenv.axon_hooks")
        mod.get_axon_ntff_profile_hook = lambda: hook
        mod.set_axon_ntff_profile_hook = lambda h: None
        sys.modules["antenv.axon_hooks"] = mod
    except Exception:
        pass


_install_ntff_shim()

import concourse.bass as bass
import concourse.mybir as mybir
import concourse.tile as tile
from concourse import bacc
from concourse.bass_utils import run_bass_kernel_spmd
from concourse.masks import make_identity

F32 = mybir.dt.float32
F32R = mybir.dt.float32r
BF16 = mybir.dt.bfloat16
EXP = mybir.ActivationFunctionType.Exp

B, S, HID = 2, 2048, 1024
HEADS, D = 16, 64
SEQ = B * S                      # 4096 flattened rows
NCORES = 8
HPC = HEADS // NCORES            # heads per core = 2
CW = HPC * D                     # per-core width = 128
NHB = HID // 128                 # hidden 128-chunks = 8
WSEQ = 512                       # seq window for QKV
NWIN = SEQ // WSEQ               # 8
QW = 1024                        # q window in attention
NKT = S // 128                   # k chunks per batch = 16
NCH = SEQ // 128                 # global 128-row chunks = 32
RING = 8


def build_nc():
    nc = bacc.Bacc("TRN2", target_bir_lowering=False, debug=False,
                   num_devices=NCORES)

    hsT = nc.dram_tensor("hsT", [HID, SEQ], BF16, kind="ExternalInput")
    w3 = nc.dram_tensor("w3", [HID, 3 * CW], BF16, kind="ExternalInput")
    bq = nc.dram_tensor("bq", [CW, 1], F32, kind="ExternalInput")
    wd = nc.dram_tensor("wd", [CW, HID], F32, kind="ExternalInput")
    out = nc.dram_tensor("out", [SEQ, HID], BF16, kind="ExternalOutput")

    with tile.TileContext(nc) as tc:
        with (
            tc.tile_pool(name="persist", bufs=1) as pp,
            tc.tile_pool(name="pt", bufs=1) as ptp,
            tc.tile_pool(name="hsload", bufs=3) as hlp,
            tc.tile_pool(name="vtw", bufs=2) as vwp,
            tc.tile_pool(name="outst", bufs=6) as osp,
            tc.tile_pool(name="ps_st", bufs=2,
                         space=bass.MemorySpace.PSUM) as pst,
            tc.tile_pool(name="ps_pv", bufs=1,
                         space=bass.MemorySpace.PSUM) as ppv,
            tc.tile_pool(name="ps_bg", bufs=2,
                         space=bass.MemorySpace.PSUM) as pbg,
        ):
            # first hs window DMA goes out before the weight loads so the
            # PE can start as early as possible
            hsTd = hsT.ap().rearrange("(c p) s -> p c s", p=128)
            hsw0 = hlp.tile([128, NHB, WSEQ], BF16)
            # window DMAs are split across two trigger queues (sync +
            # vector) so half a window can land in half the time; window 0
            # is quartered so the very first matmul chain starts ~1us in
            for qtr in range(4):
                eng = nc.sync if qtr % 2 == 0 else nc.gpsimd
                eng.dma_start(hsw0[:, 2 * qtr:2 * qtr + 2, :],
                              hsTd[:, 2 * qtr:2 * qtr + 2, 0:WSEQ])

            def dma_window(hsw, w):
                wsl = slice(w * WSEQ, (w + 1) * WSEQ)
                nc.sync.dma_start(hsw[:, 0:NHB // 2, :],
                                  hsTd[:, 0:NHB // 2, wsl])
                nc.gpsimd.dma_start(hsw[:, NHB // 2:NHB, :],
                                     hsTd[:, NHB // 2:NHB, wsl])

            ident = pp.tile([128, 128], F32)
            make_identity(nc, ident[:])
            identr_t = pp.tile([128, 128], F32R)
            nc.vector.tensor_copy(identr_t[:], ident[:])
            identr = identr_t[:]

            w3_sb = pp.tile([128, NHB, 3 * CW], BF16)
            w3d = w3.ap().rearrange("(c p) m -> p c m", p=128)
            # q columns first (split so the first stationary lands fast):
            # they are all the first matmul chain needs
            nc.gpsimd.dma_start(w3_sb[:, 0:NHB // 2, 0:CW],
                                w3d[:, 0:NHB // 2, 0:CW])
            nc.gpsimd.dma_start(w3_sb[:, NHB // 2:NHB, 0:CW],
                                w3d[:, NHB // 2:NHB, 0:CW])
            nc.gpsimd.dma_start(
                w3_sb[:, :, CW:3 * CW], w3d[:, :, CW:3 * CW])
            bq_sb = pp.tile([CW, 1], F32)
            nc.gpsimd.dma_start(bq_sb[:], bq[:])

            # Per-head q/k operands are zero-padded to a full 128-partition
            # contraction: QTz/KTz [:, h, :] rows 0-63 = head h, rows
            # 64-127 = 0.  ctxT packs head 0 in rows 0-63 and head 1 in
            # rows 64-127 (no padding needed).
            QTz = pp.tile([128, HPC, SEQ], BF16)
            KTz = pp.tile([128, HPC, SEQ], BF16)
            Vn = pp.tile([128, NCH, HPC, 66], BF16)
            ctxT = pp.tile([128, SEQ], F32R)
            den2 = pp.tile([1, HPC, QW], F32R)   # partition-0 rowsum rows
            bcden = pp.tile([128, HPC, QW], F32)  # broadcast 1/rowsums
            ones_row = pp.tile([1, 128], F32R)    # bcast matmul weights
            PT = ptp.tile([128, RING, QW], BF16)

            # ones column used by the P@V matmul to emit row sums
            ones_st = pp.tile([128, NCH * HPC], F32)
            nc.vector.memset(ones_st[:], 1.0)
            nc.vector.tensor_copy(
                Vn[:, :, :, 64:65],
                ones_st[:].rearrange("p (c h) -> p c h", c=NCH)
                .rearrange("p c h -> p c h ()"))
            ones_f = pp.tile([1, 128], F32)
            nc.vector.memset(ones_f[:], 1.0)
            nc.vector.tensor_copy(ones_row[:], ones_f[:])

            # zero-fill the padded q/k halves on the (early-idle) vector
            # engine
            zs = pp.tile([D, SEQ // 4], BF16)
            nc.vector.memset(zs[:], 0.0)
            for z0 in range(0, SEQ, SEQ // 4):
                zl = slice(z0, z0 + SEQ // 4)
                for h in range(HPC):
                    nc.vector.tensor_copy(QTz[D:128, h, zl], zs[:])
                    nc.vector.tensor_copy(KTz[D:128, h, zl], zs[:])

            # dense weights are first needed ~90us in; keep their DMA out
            # of the startup critical path
            wd_sb = pp.tile([CW, HID], F32R)
            nc.gpsimd.dma_start(wd_sb[:], wd.ap().bitcast(F32R))

            # ---------------- QKV projection window (generator) ----------
            def window_steps(w, hsw):
                """Yields between PE matmul groups so the window can be
                interleaved into the attention loop's slack."""
                r0 = w * WSEQ
                wsl = slice(r0, r0 + WSEQ)
                for tgt in range(3):
                    ps = pbg.tile([128, WSEQ], F32, tag="bg", name="qkvps")
                    wslc = w3_sb[:, :, tgt * CW:(tgt + 1) * CW]
                    for hb in range(NHB):
                        nc.tensor.matmul(
                            ps[:], wslc[:, hb, :], hsw[:, hb, :],
                            start=(hb == 0), stop=(hb == NHB - 1))
                        if hb % 2 == 1:
                            yield
                    if tgt == 0:
                        for h in range(HPC):
                            nc.vector.tensor_scalar_add(
                                QTz[0:D, h, wsl],
                                ps[h * D:(h + 1) * D, :],
                                bq_sb[h * D:(h + 1) * D, 0:1])
                    elif tgt == 1:
                        # k-bias shifts every logit of a q-row equally;
                        # softmax is invariant, so it is dropped (exact)
                        for h in range(HPC):
                            nc.vector.tensor_copy(
                                KTz[0:D, h, wsl],
                                ps[h * D:(h + 1) * D, :])
                    else:
                        vtw = vwp.tile([128, WSEQ], F32R)
                        nc.vector.tensor_copy(vtw[:], ps[:])
                        yield
                        vps = pbg.tile([128, WSEQ], F32, tag="bg",
                                       name="vps")
                        for sb2 in range(WSEQ // 128):
                            nc.tensor.transpose(
                                vps[:, sb2 * 128:(sb2 + 1) * 128]
                                .bitcast(F32R),
                                vtw[:, sb2 * 128:(sb2 + 1) * 128],
                                identr)
                            if sb2 == 1:
                                yield
                        ch0 = r0 // 128
                        nc.vector.tensor_copy(
                            Vn[:, ch0:ch0 + 4, :, 0:64],
                            vps[:].rearrange("p (c h d) -> p c h d",
                                             c=4, h=HPC))
                    yield

            # windows 0-3 (batch 0) run up front
            for w in range(4):
                if w == 0:
                    hsw = hsw0
                else:
                    hsw = hlp.tile([128, NHB, WSEQ], BF16, name="hsw")
                    dma_window(hsw, w)
                for _ in window_steps(w, hsw):
                    pass

            # windows 4-7 (batch 1) are injected into batch 0's attention
            winq = deque()

            def queue_windows(ws):
                for w in ws:
                    hsw = hlp.tile([128, NHB, WSEQ], BF16, name="hsw")
                    dma_window(hsw, w)
                    winq.append(window_steps(w, hsw))

            queue_windows((4, 5))

            def inject_window():
                while winq:
                    try:
                        next(winq[0])
                        return True
                    except StopIteration:
                        winq.popleft()
                return False

            # ---------------- attention + output projection --------------
            def dense_steps(qbase, evict_split=False):
                """Generator: 16 micro-steps for one block's (merged
                two-head) dense output projection.  In the final flush
                (evict_split) the evictions alternate between Scalar
                and Vector, and the PSUM tiles also rotate through the
                (now idle) score pool, so the tail drains at twice the
                pipeline width."""
                i = 0
                for stl in range(QW // 128):
                    st = qbase // 128 + stl
                    ssl = slice(st * 128, (st + 1) * 128)
                    for nt in range(HID // 512):
                        nsl = slice(nt * 512, (nt + 1) * 512)
                        if evict_split and i % 2 == 1:
                            psf = pst.tile([128, QW], F32, tag="st")
                            psd = psf[:, 0:512]
                        else:
                            psf = pbg.tile([128, 512], F32, tag="bg",
                                           name="psd")
                            psd = psf[:]
                        nc.tensor.matmul(
                            psd, ctxT[:, ssl], wd_sb[:, nsl],
                            start=True, stop=True)
                        ob = osp.tile([128, 512], BF16)
                        if evict_split and i % 2 == 0:
                            nc.scalar.copy(ob[:], psd)
                        else:
                            nc.vector.tensor_copy(ob[:], psd)
                        if evict_split and i % 2 == 1:
                            nc.gpsimd.dma_start(out[ssl, nsl], ob[:])
                        else:
                            nc.sync.dma_start(out[ssl, nsl], ob[:])
                        i += 1
                        yield

            def norm_chain(qbase, h):
                """Normalize head h's ctx block, one 512-half at a
                time (broadcast rowsums with a 1-row PE matmul, wide
                approximate reciprocal, in-place multiply) so the
                first half's consumers can start sooner."""
                for half in range(QW // 512):
                    hsl = slice(half * 512, (half + 1) * 512)
                    csl = slice(qbase + half * 512,
                                qbase + (half + 1) * 512)
                    bcp = pbg.tile([128, 512], F32, tag="bg", name="bcp")
                    nc.tensor.matmul(
                        bcp[:], ones_row[:], den2[0:1, h, hsl],
                        start=True, stop=True)
                    nc.vector.reciprocal_approx_fast(
                        bcden[:, h, hsl], bcp[:])
                    nc.vector.tensor_mul(
                        ctxT[h * D:(h + 1) * D, csl],
                        ctxT[h * D:(h + 1) * D, csl].bitcast(F32),
                        bcden[h * D:(h + 1) * D, h, hsl])

            pending = None   # dense generator of the previous block
            norm_todo = []   # deferred per-head normalize chains
            ktg = 0          # global kt counter -> PT ring slot, so
                             # loop boundaries don't collide on a slot
            for b in range(B):
                for qw in range(S // QW):
                    if b == 0 and qw == 1:
                        queue_windows((6, 7))
                    qbase = b * S + qw * QW
                    for hh in range(HPC):
                        pva = ppv.tile([D + 1, 512], F32, tag="pva")
                        pvb = ppv.tile([D + 1, 512], F32, tag="pvb")

                        def score_exp(kt):
                            nonlocal ktg
                            ksl = slice(b * S + kt * 128,
                                        b * S + (kt + 1) * 128)
                            rg = ktg % RING
                            ktg += 1
                            stp = pst.tile([128, QW], F32, tag="st")
                            for qh in range(QW // 512):
                                sl = slice(qh * 512, (qh + 1) * 512)
                                nc.tensor.matmul(
                                    stp[:, sl], KTz[:, hh, ksl],
                                    QTz[:, hh,
                                        qbase + qh * 512:
                                        qbase + (qh + 1) * 512],
                                    start=True, stop=True)
                            nc.scalar.activation(
                                PT[:, rg, :], stp[:], EXP, scale=0.125)
                            return rg

                        def pv(kt, rg):
                            ch = b * NKT + kt
                            for qh, pvh in ((0, pva), (1, pvb)):
                                sl = slice(qh * 512, (qh + 1) * 512)
                                nc.tensor.matmul(
                                    pvh[:], Vn[:, ch, hh, 0:65],
                                    PT[:, rg, sl],
                                    start=(kt == 0),
                                    stop=(kt == NKT - 1))

                        # peel kts 0-1: both their scores+EXPs issue
                        # before any PV, so the PV(kt 0) stall on the
                        # previous head's PSUM eviction cannot
                        # interrupt the scalar engine's EXP stream.
                        # The normalize chain of the previous head
                        # rides in the same shadow.
                        rg0 = score_exp(0)
                        rg1 = score_exp(1)
                        pv(0, rg0)
                        pv(1, rg1)
                        for kt in range(2, NKT):
                            rg = score_exp(kt)
                            pv(kt, rg)
                            # deferred work rides the scalar-bound
                            # loop: the previous head's normalize
                            # chain at kt 2, then pending QKV windows
                            # (batch 1) and the previous block's dense
                            # steps in the remaining slack
                            if kt == 2 and norm_todo:
                                norm_chain(*norm_todo.pop(0))
                            elif winq:
                                inject_window()
                                if kt % 2 == 0 and kt >= 8:
                                    inject_window()
                            elif pending is not None and kt >= 3:
                                next(pending, None)
                        # den+ctx per tile, pva fully first, so the
                        # next loop's PV(kt 0) unblocks sooner
                        for qh, pvh in ((0, pva), (1, pvb)):
                            dsl = slice(qh * 512, (qh + 1) * 512)
                            nc.vector.tensor_copy(
                                den2[0:1, hh, dsl], pvh[D:D + 1, :])
                            s2 = slice(qbase + qh * 512,
                                       qbase + (qh + 1) * 512)
                            nc.vector.tensor_copy(
                                ctxT[hh * D:(hh + 1) * D, s2],
                                pvh[0:D, :])
                        norm_todo.append((qbase, hh))
                    # drain dense leftovers of the previous block
                    if pending is not None:
                        for _ in pending:
                            pass
                    last = (b == B - 1 and qw == S // QW - 1)
                    pending = dense_steps(qbase, evict_split=last)
            # flush: remaining windows (shouldn't happen), last head's
            # normalize + last block's dense
            while winq:
                inject_window()
            while norm_todo:
                norm_chain(*norm_todo.pop(0))
            for _ in pending:
                pass

    nc.compile()
    return nc


_NC_CACHE = None


def get_nc():
    global _NC_CACHE
    if _NC_CACHE is None:
        _NC_CACHE = build_nc()
    return _NC_CACHE


def make_in_maps(hidden_states, w_qkv, b_qkv, w_dense):
    hs = np.asarray(hidden_states, dtype=np.float32).reshape(SEQ, HID)
    hsT = np.ascontiguousarray(hs.T).astype(BF16_NP)
    w_qkv = np.asarray(w_qkv, dtype=np.float32)
    b_qkv = np.asarray(b_qkv, dtype=np.float32)
    w_dense = np.asarray(w_dense, dtype=np.float32)
    # Reference layout: qkv.reshape(B, S, HEADS, 3*D) split on the last
    # axis, i.e. w_qkv columns are per-head [q_h | k_h | v_h] blocks of D.
    wq_cols = np.concatenate(
        [np.arange(h * 3 * D, h * 3 * D + D) for h in range(HEADS)])
    wk_cols = wq_cols + D
    wv_cols = wq_cols + 2 * D
    in_maps = []
    for c in range(NCORES):
        c0 = c * CW
        sel = slice(c0, c0 + CW)
        w3 = np.concatenate(
            [w_qkv[:, wq_cols[sel]], w_qkv[:, wk_cols[sel]],
             w_qkv[:, wv_cols[sel]]], axis=1).astype(BF16_NP)
        in_maps.append({
            "hsT": hsT,
            "w3": np.ascontiguousarray(w3),
            "bq": np.ascontiguousarray(b_qkv[wq_cols[sel]].reshape(CW, 1)),
            "wd": np.ascontiguousarray(w_dense[sel, :]),
        })
    return in_maps


def run(hidden_states, w_qkv, b_qkv, w_dense, b_dense, trace=False):
    nc = get_nc()
    in_maps = make_in_maps(hidden_states, w_qkv, b_qkv, w_dense)
    res = run_bass_kernel_spmd(nc, in_maps, core_ids=list(range(NCORES)),
                               trace=trace)
    acc = res.results[0]["out"].astype(np.float32)
    for c in range(1, NCORES):
        acc = acc + res.results[c]["out"]
    # bias terms that commute to the end: v-bias through dense, dense bias
    b_qkv = np.asarray(b_qkv, dtype=np.float32)
    b_v = np.concatenate(
        [b_qkv[h * 3 * D + 2 * D:h * 3 * D + 3 * D] for h in range(HEADS)])
    acc = acc + (b_v @ np.asarray(w_dense, dtype=np.float32)
                 + np.asarray(b_dense, dtype=np.float32))
    return acc.reshape(B, S, HID).astype(np.float32), res


def kernel(hidden_states, w_qkv, b_qkv, w_dense, b_dense):
    out, _ = run(hidden_states, w_qkv, b_qkv, w_dense, b_dense,
                 trace=bool(os.environ.get("BASS_TRACE")))
    return out
